# revision 10
# baseline (speedup 1.0000x reference)
"""APDL_RNN (allpass-delayed LSTM) Trainium2 kernel, 8-core data-parallel.

Problem: B=32, T=8192, I=1, H=256. Strictly sequential scan over T with an
LSTM cell whose (h, c) input state is an allpass-interpolated delayed state:
    ap(t)   = (1/3)*(s(t-1) - ap(t-1)) + s(t-2)
    gates   = x(t)@W_ih.T + b_ih + ap_h(t)@W_hh.T + b_hh   (i,f,g,o torch order)
    c_new   = sig(f)*ap_c + sig(i)*tanh(g)
    h_new   = sig(o)*tanh(c_new)
    s(t)    = [h_new, c_new]

Sharding: data-parallel over batch, 4 sequences per NeuronCore.

Layout (per core): feature-major. Every per-step tensor lives as
[128 partitions = feature%128, free = (feature_tile, batch)], so elementwise
ops have tiny free dims (8-32) and use all 128 lanes. State tile order is
(h0, h1, c0, c1); gate tile order (i0,i1,f0,f1,o0,o1,g0,g1) so sigmoid runs on
one contiguous 24-col block and tanh on one 8-col block. The recurrent matmul
keeps W_hh tiles stationary in the PE array (16 LDW+matmul pairs per step,
K=2x128 accumulated in PSUM, N=4). xg = x*W_ih + b_ih + b_hh is generated
on-the-fly per 128-step chunk by GPSIMD (rank-1, per-partition affine).
States accumulate in an SBUF ring and are DMA'd out once per chunk.
"""

import numpy as np
import ml_dtypes

import concourse.bass as bass
import concourse.bacc as bacc
import concourse.tile as tile
from concourse import mybir
from concourse.bass_utils import run_bass_kernel_spmd

BF16 = ml_dtypes.bfloat16

B, T, H = 32, 8192, 256
NCORES = 8
BL = B // NCORES          # 4 sequences per core
G = 4 * H                 # 1024 gate features
P = 128
NKT = H // P              # 2 contraction tiles
NGT = G // P              # 8 gate tiles
SCOLS = 4 * BL            # 16 state cols per step  (h0,h1,c0,c1) x batch
GCOLS = NGT * BL          # 32 gate cols per step
CHUNK = 128               # timesteps per For_i iteration
NCH = T // CHUNK
COEFF = float(1.0 / 3.0)

# gate row permutation: torch (i,f,g,o) -> our tile order (i,f,o,g)
_PERM = np.concatenate([
    np.arange(0, 256), np.arange(256, 512), np.arange(768, 1024), np.arange(512, 768)
])


def _build_graph(chunk=CHUNK, nch=NCH, dbg=False):
    tt = chunk * nch  # timesteps this graph executes
    nc = bacc.Bacc("TRN2", target_bir_lowering=False)
    fp32 = mybir.dt.float32
    bf16 = mybir.dt.bfloat16

    xl_d = nc.declare_dram_parameter("xl", [T * 16], fp32, isOutput=False)
    wsb_d = nc.declare_dram_parameter("wsb", [P, NGT * NKT * P], bf16, isOutput=False)
    wib_d = nc.declare_dram_parameter("wib", [P, 16], fp32, isOutput=False)
    out_d = nc.declare_dram_parameter("out", [P, tt * SCOLS], bf16, isOutput=True)
    apf_d = nc.declare_dram_parameter("apf", [P, SCOLS], bf16, isOutput=True)
    if dbg:
        dxgb_d = nc.declare_dram_parameter("dxgb", [P, chunk * GCOLS], fp32, isOutput=True)
        dgat_d = nc.declare_dram_parameter("dgat", [P, chunk * GCOLS], bf16, isOutput=True)

    with tile.TileContext(nc) as tc:
        with (
            tc.tile_pool(name="const", bufs=1) as constp,
            tc.tile_pool(name="state", bufs=1) as statep,
            tc.tile_pool(name="xchunk", bufs=1) as xchunkp,
            tc.tile_pool(name="step", bufs=3) as stepp,
            tc.tile_pool(name="psum", bufs=4, space="PSUM") as psump,
        ):
            wsb = constp.tile([P, NGT * NKT * P], bf16)
            wib = constp.tile([P, 16], fp32)
            nc.sync.dma_start(wsb[:], wsb_d[:])
            nc.sync.dma_start(wib[:], wib_d[:])

            # state ring = output ring; also holds s1/s2 history
            sbuf = statep.tile([P, chunk * SCOLS], bf16)
            ap = statep.tile([P, SCOLS], bf16)
            nc.vector.memset(sbuf[:], 0.0)
            nc.vector.memset(ap[:], 0.0)

            xrep = xchunkp.tile([P, chunk * 16], fp32)
            xgb = xchunkp.tile([P, chunk * GCOLS], fp32)

            with tc.For_i(0, tt * SCOLS, chunk * SCOLS) as iv:
                # ---- per-chunk prologue: xgb generation ----
                nc.sync.dma_start(
                    xrep[:], xl_d[bass.ds(iv, chunk * 16)].partition_broadcast(P)
                )
                xr3 = xrep[:].rearrange("p (t s) -> p t s", s=16)[:, :, 0:BL]
                xg3 = xgb[:].rearrange("p (t g) -> p t g", g=GCOLS)
                for gt in range(NGT):
                    nc.gpsimd.tensor_scalar(
                        xg3[:, :, gt * BL:(gt + 1) * BL],
                        xr3,
                        wib[:, gt:gt + 1],
                        wib[:, 8 + gt:8 + gt + 1],
                        mybir.AluOpType.mult,
                        mybir.AluOpType.add,
                    )
                if dbg:
                    nc.sync.dma_start(dxgb_d[:], xgb[:])

                # ---- recurrent steps ----
                for t in range(chunk):
                    tp = (t - 1) % chunk
                    tpp = (t - 2) % chunk
                    s1 = sbuf[:, tp * SCOLS:(tp + 1) * SCOLS]
                    s2 = sbuf[:, tpp * SCOLS:(tpp + 1) * SCOLS]

                    d = stepp.tile([P, SCOLS], bf16, tag="d")
                    nc.vector.tensor_tensor(d[:], s1, ap[:], mybir.AluOpType.subtract)
                    # ap = d/3 + s2
                    nc.vector.scalar_tensor_tensor(
                        ap[:], d[:], COEFF, s2,
                        mybir.AluOpType.mult, mybir.AluOpType.add,
                    )

                    ps = psump.tile([P, GCOLS], fp32, tag="ps")
                    for gt in range(NGT):
                        for kt in range(NKT):
                            j = gt * NKT + kt
                            nc.tensor.matmul(
                                ps[:, gt * BL:(gt + 1) * BL],
                                wsb[:, j * P:(j + 1) * P],
                                ap[:, kt * BL:(kt + 1) * BL],
                                start=(kt == 0),
                                stop=(kt == NKT - 1),
                            )

                    gsb = stepp.tile([P, GCOLS], bf16, tag="gsb")
                    nc.vector.tensor_tensor(
                        gsb[:], ps[:], xgb[:, t * GCOLS:(t + 1) * GCOLS],
                        mybir.AluOpType.add,
                    )
                    if dbg:
                        nc.sync.dma_start(
                            dgat_d[:, t * GCOLS:(t + 1) * GCOLS], gsb[:]
                        )

                    act = stepp.tile([P, GCOLS], bf16, tag="act")
                    nc.scalar.activation(
                        act[:, 0:24], gsb[:, 0:24],
                        mybir.ActivationFunctionType.Sigmoid,
                    )
                    nc.scalar.activation(
                        act[:, 24:32], gsb[:, 24:32],
                        mybir.ActivationFunctionType.Tanh,
                    )

                    snew = sbuf[:, t * SCOLS:(t + 1) * SCOLS]
                    tf = stepp.tile([P, 2 * BL], bf16, tag="tf")
                    ti = stepp.tile([P, 2 * BL], bf16, tag="ti")
                    # sig(f) * ap_c
                    nc.vector.tensor_tensor(
                        tf[:], act[:, 8:16], ap[:, 8:16], mybir.AluOpType.mult
                    )
                    # sig(i) * tanh(g)
                    nc.vector.tensor_tensor(
                        ti[:], act[:, 0:8], act[:, 24:32], mybir.AluOpType.mult
                    )
                    # c_new -> state cols 8:16
                    nc.vector.tensor_tensor(
                        snew[:, 8:16], tf[:], ti[:], mybir.AluOpType.add
                    )
                    tc_t = stepp.tile([P, 2 * BL], bf16, tag="tc")
                    nc.scalar.activation(
                        tc_t[:], snew[:, 8:16], mybir.ActivationFunctionType.Tanh
                    )
                    # h_new -> state cols 0:8
                    nc.vector.tensor_tensor(
                        snew[:, 0:8], act[:, 16:24], tc_t[:], mybir.AluOpType.mult
                    )

                # ---- chunk epilogue: flush states ----
                nc.sync.dma_start(out_d[:, bass.ds(iv, chunk * SCOLS)], sbuf[:])

            nc.sync.dma_start(apf_d[:], ap[:])

    nc.compile()
    return nc


def _prep_core_inputs(x_core, W_ih, W_hh, b_ih, b_hh):
    """Host-side packing for one core's BL sequences."""
    # xl[t*16 + b] = x[b, t]
    xl = np.zeros((T * 16,), np.float32)
    for b in range(BL):
        xl[b::16] = x_core[b, :, 0]
    # stationary W blocks: block j=gt*2+kt, [k_part, m_g]
    wsb = np.empty((P, NGT * NKT * P), BF16)
    Wp = W_hh[_PERM, :]  # [G, H] permuted rows
    for gt in range(NGT):
        for kt in range(NKT):
            j = gt * NKT + kt
            blk = Wp[gt * P:(gt + 1) * P, kt * P:(kt + 1) * P]  # [m_g, k]
            wsb[:, j * P:(j + 1) * P] = blk.T.astype(BF16)
    wib = np.empty((P, 16), np.float32)
    bias = (b_ih + b_hh)[_PERM]
    wihp = W_ih[_PERM, 0]
    for gt in range(NGT):
        wib[:, gt] = wihp[gt * P:(gt + 1) * P]
        wib[:, 8 + gt] = bias[gt * P:(gt + 1) * P]
    return {"xl": xl, "wsb": wsb, "wib": wib}


_GRAPH_CACHE = {}

# set by test harness: collect a neuron-profile trace and print HW exec time
TRACE = False
LAST_RESULT = {}


def kernel(x, W_ih, W_hh, b_ih, b_hh):
    x = np.asarray(x, np.float32)
    W_ih = np.asarray(W_ih, np.float32)
    W_hh = np.asarray(W_hh, np.float32)
    b_ih = np.asarray(b_ih, np.float32)
    b_hh = np.asarray(b_hh, np.float32)

    if "nc" not in _GRAPH_CACHE:
        _GRAPH_CACHE["nc"] = _build_graph()
    nc = _GRAPH_CACHE["nc"]

    in_maps = []
    for c in range(NCORES):
        x_core = x[c * BL:(c + 1) * BL]
        in_maps.append(_prep_core_inputs(x_core, W_ih, W_hh, b_ih, b_hh))

    kw = {}
    if TRACE:
        kw = {"trace": True}
    res = run_bass_kernel_spmd(nc, in_maps, core_ids=list(range(NCORES)), **kw)
    results = res.results
    LAST_RESULT["exec_time_ns"] = res.exec_time_ns
    LAST_RESULT["res"] = res
    if res.exec_time_ns is not None:
        print(f"HW exec time: {res.exec_time_ns} ns")

    y = np.empty((B, T, H), np.float32)
    c_out = np.empty((B, T, H), np.float32)
    ap_out = np.empty((B, 2 * H), np.float32)
    for c in range(NCORES):
        o = np.asarray(results[c]["out"]).astype(np.float32)  # [P, T*16]
        o = o.reshape(P, T, 4, BL)          # p, t, tile, b
        o = o.transpose(3, 1, 2, 0)         # b, t, tile, p
        y[c * BL:(c + 1) * BL] = o[:, :, 0:2, :].reshape(BL, T, H)
        c_out[c * BL:(c + 1) * BL] = o[:, :, 2:4, :].reshape(BL, T, H)
        a = np.asarray(results[c]["apf"]).astype(np.float32)  # [P, 16]
        a = a.reshape(P, 4, BL).transpose(2, 1, 0)            # b, tile, p
        ap_out[c * BL:(c + 1) * BL] = a.reshape(BL, 2 * H)

    return y, y, c_out, ap_out
